# revision 9
# baseline (speedup 1.0000x reference)
"""DeTPP assignment loss on Trainium2, data-parallel over batch across 8 NeuronCores.

Pipeline per core (B_shard = 8 batch columns, N_s = 512*8 = 4096 windows):
  host   : pure-index gathers (rolling windows, per-batch row selection,
           true-class logit pick), shard + pack fp16 partition-major layouts
  device : sum(exp) over C=128 (the memory-bound bulk: 4.2 MiB of gathered
           fp16 logits per core), L1/CE cost assembly, exact 24-permutation
           assignment min via pair-sum decomposition, softplus leftover,
           mask-weighted reduction to one scalar (partition-sum on PE)
  host   : sum 8 core scalars / V

Key algebra: cost[k,t] = base[k,t] + (lse_k - ps_k) with
base = |ot-tt| + |oa-at| - logit[true class]; the (lse_k - ps_k) part is
independent of the assignment, so the 24-perm min runs on `base` alone and
sum_k lse_k + sum_k softplus(ps_k) = ln(prod_k se_k * prod_k (1+e^{ps_k}))
needs a single Ln per window.

Measured hardware model (perfetto): the input stream runs at ~200 GB/s
effective, so the kernel is DMA-paced; ACT exp (~1 col/cycle) hides in the
stream's shadow. Each logits chunk is its own contiguous DRAM tensor so the
HBM reads are fully sequential. All elementwise tiles are flat (P, W) so
DVE runs in packed-fp16 fast mode (4D tiles execute as 4-element strided
runs, ~3x slower). The per-window ln runs per chunk, so after the last DMA
byte only one small chunk's worth of work trails. The final scalar is
partition-summed on the otherwise-idle PE so the output DMA moves 4 bytes
on one queue: one completion event instead of 16 (completion events drain
at ~340ns each and previously added ~6us to the epilogue). One combined
exp+ln act table (set 6) is loaded once up front.
"""
import numpy as np

L, B, K, C = 2048, 64, 4, 128
I = 512
NCORES = 8
BS = B // NCORES          # batch columns per core
NS = I * BS               # windows per core
P = 128                   # partitions
NT = NS // P              # 32 row-tiles per core
KC = K * C                # 512

# tiles per logits DMA chunk: tiny head (fast ramp), big middle, small drain
CHUNKS = [1, 2, 4, 8, 8, 6, 2, 1]
assert sum(CHUNKS) == NT

# small-tensor column offsets within the packed (P, SMW) fp16 tensor
OFF_OLT, OFF_OT, OFF_TT, OFF_OA, OFF_AT, OFF_PS, OFF_M, SMW = \
    0, 512, 640, 768, 896, 1024, 1152, 1184

# unordered window pairs; split q assigns A-pair PAIRS[SPLITS[q][0]] to
# outputs (k0,k1) and the complementary B-pair PAIRS[SPLITS[q][1]] to (k2,k3)
PAIRS = [(0, 1), (2, 3), (0, 2), (1, 3), (0, 3), (1, 2)]
SPLITS = [(0, 1), (1, 0), (2, 3), (3, 2), (4, 5), (5, 4)]

_PROGRAM = None


def _prep(in_time, in_amount, in_mcc, out_time, out_amount, out_logits,
          presence, lengths, indices, subset_lengths):
    """Host-side pure-index gather, mirroring reference _windows/_select."""
    f = np.float32
    idx = np.clip(np.asarray(indices), 0, L - 1)            # (I, B)
    br = np.arange(B)[None, :]
    win = (idx[:, :, None] + np.arange(K + 1)[None, None, :]) % L
    bw = br[:, :, None]
    tw = np.asarray(in_time)[win, bw].astype(f)             # (I,B,K+1)
    aw = np.asarray(in_amount)[win, bw].astype(f)
    cw = np.clip(np.asarray(in_mcc)[win, bw], 0, C - 1)     # (I,B,K+1)
    t_true = tw[..., 1:] - tw[..., :1]                      # (I,B,K)
    a_true = aw[..., 1:]
    true_c = cw[..., 1:]
    lg = np.asarray(out_logits)[idx, br].astype(f)          # (I,B,K,C)
    ol_true = np.take_along_axis(lg, true_c[:, :, None, :], axis=3)  # (I,B,K,T)
    ot = np.asarray(out_time)[idx, br].astype(f)            # (I,B,K)
    oa = np.asarray(out_amount)[idx, br].astype(f)
    ps = np.asarray(presence)[idx, br].astype(f)
    m = (np.arange(I)[:, None] < np.asarray(subset_lengths)[None, :]).astype(f)
    return dict(lg=lg, ol_true=ol_true, ot=ot, t_true=t_true, oa=oa,
                a_true=a_true, ps=ps, m=m)


def _pack_core(g, d):
    """Shard batch columns [d*BS, (d+1)*BS) and pack partition-major fp16:
    row n = i*BS + b_local lives at (tile j = n//P, partition p = n%P).
    The logits are split into per-chunk contiguous DRAM tensors so every
    chunk DMA is a fully sequential HBM read."""
    sl = slice(d * BS, (d + 1) * BS)

    def pk(a):
        w = int(np.prod(a.shape[2:], dtype=np.int64)) if a.ndim > 2 else 1
        return a[:, sl].reshape(NT, P, w).transpose(1, 0, 2).reshape(P, NT * w)

    small = np.concatenate(
        [pk(g["ol_true"]), pk(g["ot"]), pk(g["t_true"]), pk(g["oa"]),
         pk(g["a_true"]), pk(g["ps"]), pk(g["m"])], axis=1).astype(np.float16)
    assert small.shape == (P, SMW)
    logits = pk(g["lg"]).astype(np.float16)
    out = {"small": small}
    off = 0
    for ci, t in enumerate(CHUNKS):
        out[f"lg{ci}"] = np.ascontiguousarray(
            logits[:, off * KC:(off + t) * KC])
        off += t
    return out


def _build_program(debug=False):
    import concourse.bacc as bacc
    import concourse.tile as tile
    import concourse.mybir as mybir

    f32 = mybir.dt.float32
    f16 = mybir.dt.float16
    AF = mybir.ActivationFunctionType
    ALU = mybir.AluOpType
    AX = mybir.AxisListType.X

    nc = bacc.Bacc("TRN2", target_bir_lowering=False, debug=debug)
    lg_ds = [nc.dram_tensor(f"lg{ci}", [P, t * KC], f16, kind="ExternalInput")
             for ci, t in enumerate(CHUNKS)]
    sm_d = nc.dram_tensor("small", [P, SMW], f16, kind="ExternalInput")
    out_d = nc.dram_tensor("partial", [1, 1], f32, kind="ExternalOutput")

    NW = NT * K * K           # 512: flat width of (NT, K, K) tensors

    with tile.TileContext(nc) as tc:
        with tc.tile_pool(name="big", bufs=1) as big, \
             tc.tile_pool(name="res", bufs=1) as res, \
             tc.psum_pool(name="pacc", bufs=1) as pacc:

            def rtile(tag, shape, dt=f16):
                return res.tile(list(shape), dt, tag=tag, name=tag)

            # combined exp+ln table (set 6 = natural_log_exp_and_others)
            # loaded once up front, overlapped with the first DMA
            nc.scalar.add_instruction(mybir.InstLoadActFuncSet(
                name=nc.get_next_instruction_name(), ins=[], outs=[],
                act_func_set_id=6))

            # first logits chunk DMA goes out first (it gates the ACT
            # stream); the small tensor second (GpSimd chain has slack)
            lgs = []
            for ci, t in enumerate(CHUNKS):
                lg = big.tile([P, t * KC], f16, tag=f"lg{ci}", name=f"lg{ci}")
                lgs.append(lg)
                nc.sync.dma_start(out=lg[:], in_=lg_ds[ci].ap())
                if ci == 0:
                    sm = rtile("sm", (P, SMW))
                    nc.sync.dma_start(out=sm[:], in_=sm_d.ap())

            ones = rtile("ones", (P, 1), f32)
            nc.vector.memset(ones[:], 1.0)
            olt = sm[:, OFF_OLT:OFF_OT]                     # flat (P, 512)
            ot4 = sm[:, OFF_OT:OFF_TT].rearrange("p (j a) -> p j a", a=K)
            tt4 = sm[:, OFF_TT:OFF_OA].rearrange("p (j a) -> p j a", a=K)
            oa4 = sm[:, OFF_OA:OFF_AT].rearrange("p (j a) -> p j a", a=K)
            at4 = sm[:, OFF_AT:OFF_PS].rearrange("p (j a) -> p j a", a=K)
            ps4 = sm[:, OFF_PS:OFF_M].rearrange("p (j a) -> p j a", a=K)
            m1 = sm[:, OFF_M:SMW]

            TS = (P, NT, K, K)

            def flat(tag):
                # flat (P, 512) tile + a 4D (P, NT, K, K) view of it; flat
                # APs keep DVE in packed-fp16 fast mode
                tl = rtile(tag, (P, NW))
                return tl, tl[:].rearrange("p (j a b) -> p j a b", a=K, b=K)

            # --- base[n,k,t] = |ot-tt| + |oa-at| - olt: broadcast-heavy
            # subs on GpSimd, everything contiguous finished on DVE in
            # fast mode ---
            d_t, d_t4 = flat("d_t")
            nc.gpsimd.tensor_sub(d_t4, ot4.unsqueeze(3).broadcast_to(TS),
                                 tt4.unsqueeze(2).broadcast_to(TS))
            d_tn, d_tn4 = flat("d_tn")
            nc.gpsimd.tensor_sub(d_tn4, tt4.unsqueeze(2).broadcast_to(TS),
                                 ot4.unsqueeze(3).broadcast_to(TS))
            d_a, d_a4 = flat("d_a")
            nc.gpsimd.tensor_sub(d_a4, oa4.unsqueeze(3).broadcast_to(TS),
                                 at4.unsqueeze(2).broadcast_to(TS))
            d_an, d_an4 = flat("d_an")
            nc.gpsimd.tensor_sub(d_an4, at4.unsqueeze(2).broadcast_to(TS),
                                 oa4.unsqueeze(3).broadcast_to(TS))
            nc.vector.tensor_max(d_t[:], d_t[:], d_tn[:])
            nc.vector.tensor_max(d_a[:], d_a[:], d_an[:])
            base, base4 = flat("base")
            nc.vector.tensor_add(base[:], d_t[:], d_a[:])
            nc.vector.tensor_sub(base[:], base[:], olt)

            # pair sums A[t0,t1] = base[k0,t0]+base[k1,t1] (B for k2,k3),
            # plus explicit transposes so the unordered-pair min is a
            # contiguous fast-mode DVE op
            b0 = base4[:, :, 0, :]
            b1 = base4[:, :, 1, :]
            b2 = base4[:, :, 2, :]
            b3 = base4[:, :, 3, :]
            A, A4 = flat("A")
            nc.gpsimd.tensor_add(A4, b0.unsqueeze(3).broadcast_to(TS),
                                 b1.unsqueeze(2).broadcast_to(TS))
            At, At4 = flat("At")
            nc.gpsimd.tensor_add(At4, b0.unsqueeze(2).broadcast_to(TS),
                                 b1.unsqueeze(3).broadcast_to(TS))
            Bp, Bp4 = flat("Bp")
            nc.gpsimd.tensor_add(Bp4, b2.unsqueeze(3).broadcast_to(TS),
                                 b3.unsqueeze(2).broadcast_to(TS))
            Bt, Bt4 = flat("Bt")
            nc.gpsimd.tensor_add(Bt4, b2.unsqueeze(2).broadcast_to(TS),
                                 b3.unsqueeze(3).broadcast_to(TS))

            # leftover pieces (tiny): e4 = exp(ps)+1, qe = prod_k e4,
            # pss = sum_k ps (pairwise; GpSimd can't free-dim reduce)
            e4 = rtile("e4", (P, NT, K))
            nc.scalar.activation(out=e4[:], in_=ps4, func=AF.Exp)
            nc.gpsimd.tensor_add(e4[:], e4[:],
                                 ones[:].unsqueeze(2).broadcast_to((P, NT, K)))
            q1 = rtile("q1", (P, NT, 2))
            nc.gpsimd.tensor_mul(q1[:], e4[:, :, 0:2], e4[:, :, 2:4])
            qe = rtile("qe", (P, NT), f32)
            nc.gpsimd.tensor_mul(qe[:], q1[:, :, 0], q1[:, :, 1])
            ps1 = rtile("ps1", (P, NT, 2), f32)
            nc.gpsimd.tensor_add(ps1[:], ps4[:, :, 0:2], ps4[:, :, 2:4])
            pss = rtile("pss", (P, NT), f32)
            nc.gpsimd.tensor_add(pss[:], ps1[:, :, 0], ps1[:, :, 1])

            mA, mA4 = flat("mA")
            mB, mB4 = flat("mB")
            pmin = rtile("pmin", (P, NT), f32)
            V6 = rtile("V6", (P, NT, 6))
            tot0 = rtile("tot0", (P, NT), f32)

            def emit_mins():
                # contiguous fp16 mins (2x fast mode) over the explicit
                # transposes; emitted mid-chunk-loop so they land on DVE
                # when A/At/Bp/Bt (GpSimd) are done
                nc.vector.tensor_tensor(out=mA[:], in0=A[:], in1=At[:],
                                        op=ALU.min)
                nc.vector.tensor_tensor(out=mB[:], in0=Bp[:], in1=Bt[:],
                                        op=ALU.min)
                for q, (ja, jb) in enumerate(SPLITS):
                    a0, a1 = PAIRS[ja]
                    c0, c1 = PAIRS[jb]
                    nc.gpsimd.tensor_add(V6[:, :, q], mA4[:, :, a0, a1],
                                         mB4[:, :, c0, c1])

            # --- per chunk: exp (ACT) -> halving-tree sums (DVE, packed
            # fp16 2x) -> qs -> qq (DVE) -> ln (ACT) -> masked total (DVE);
            # chunks cover whole windows so everything completes per chunk
            # and only the last (1-tile) chunk trails the final DMA ---
            se_all = rtile("se_all", (P, NT, K))
            qq = rtile("qq", (P, NT), f32)
            lnq = rtile("lnq", (P, NT), f32)
            totm = rtile("totm", (P, NT), f32)
            off = 0
            for ci, t in enumerate(CHUNKS):
                lg = lgs[ci]
                nc.scalar.activation(out=lg[:], in_=lg[:], func=AF.Exp)
                g = t * K
                v = lg[:].rearrange("p (g c) -> p g c", c=C)
                h1 = big.tile([P, g, 64], f16, tag=f"h1_{ci}", name=f"h1_{ci}")
                nc.vector.tensor_add(h1[:], v[:, :, 0:64], v[:, :, 64:128])
                h2 = big.tile([P, g, 32], f16, tag=f"h2_{ci}", name=f"h2_{ci}")
                nc.vector.tensor_add(h2[:], h1[:, :, 0:32], h1[:, :, 32:64])
                h3 = big.tile([P, g, 16], f16, tag=f"h3_{ci}", name=f"h3_{ci}")
                nc.vector.tensor_add(h3[:], h2[:, :, 0:16], h2[:, :, 16:32])
                se = se_all[:, off:off + t, :]
                with nc.allow_low_precision(reason="sumexp fits fp16"):
                    nc.vector.tensor_reduce(out=se, in_=h3[:], axis=AX,
                                            op=ALU.add)
                # qs = prod_k se_k, qq = qs * qe for this chunk's windows
                qsc = big.tile([P, t, 2], f32, tag=f"qs_{ci}", name=f"qs_{ci}")
                nc.vector.tensor_mul(qsc[:], se[:, :, 0:2], se[:, :, 2:4])
                nc.vector.tensor_mul(qq[:, off:off + t], qsc[:, :, 0],
                                     qsc[:, :, 1])
                nc.vector.tensor_mul(qq[:, off:off + t], qq[:, off:off + t],
                                     qe[:, off:off + t])
                nc.scalar.activation(out=lnq[:, off:off + t],
                                     in_=qq[:, off:off + t], func=AF.Ln)
                off += t
                if ci == 2:
                    emit_mins()
                if ci == 3:
                    # min over the 6 split assignments, then fold -pss on
                    # GpSimd: tot0 = pmin - pss (well before the tail)
                    nc.vector.tensor_reduce(out=pmin[:], in_=V6[:], axis=AX,
                                            op=ALU.min)
                    nc.gpsimd.tensor_sub(tot0[:], pmin[:], pss[:])

            # bulk finalize: (lnq + tot0) * m, then rowsum -- three cheap
            # flat ops on the tail
            nc.vector.tensor_add(lnq[:], lnq[:], tot0[:])
            nc.vector.tensor_mul(totm[:], lnq[:], m1)
            rowsum = rtile("rowsum", (P, 1), f32)
            nc.vector.tensor_reduce(out=rowsum[:], in_=totm[:], axis=AX,
                                    op=ALU.add)
            # partition-sum on PE (ones^T @ rowsum) so the output DMA is a
            # single 4-byte packet on one queue: one completion event
            acc = pacc.tile([1, 1], f32, tag="acc", name="acc")
            nc.tensor.matmul(out=acc[:], lhsT=ones[:], rhs=rowsum[:],
                             start=True, stop=True)
            scl = rtile("scl", (1, 1), f32)
            nc.vector.tensor_copy(out=scl[:], in_=acc[:])
            nc.sync.dma_start(out=out_d.ap(), in_=scl[:], single_packet=True)

    nc.compile()
    return nc


def _get_program():
    global _PROGRAM
    if _PROGRAM is None:
        _PROGRAM = _build_program()
    return _PROGRAM


def kernel(**inputs):
    g = _prep(**inputs)
    in_maps = [_pack_core(g, d) for d in range(NCORES)]
    nc = _get_program()
    from concourse.bass_utils import run_bass_kernel_spmd
    res = run_bass_kernel_spmd(nc, in_maps, list(range(NCORES)))
    total = sum(float(r["partial"][0, 0]) for r in res.results)
    V = g["m"].sum(dtype=np.float64)
    return np.asarray(np.float32(total) / np.float32(V))


# revision 10
# speedup vs baseline: 1.0695x; 1.0695x over previous
"""DeTPP assignment loss on Trainium2, data-parallel over batch across 8 NeuronCores.

Pipeline per core (B_shard = 8 batch columns, N_s = 512*8 = 4096 windows):
  host   : pure-index gathers (rolling windows, per-batch row selection,
           true-class logit pick), shard + pack fp16 partition-major layouts
  device : sum(exp) over C=128 (the memory-bound bulk: 4.2 MiB of gathered
           fp16 logits per core), L1/CE cost assembly, exact 24-permutation
           assignment min via pair-sum decomposition, softplus leftover,
           mask-weighted reduction to one scalar (partition-sum on PE)
  host   : sum 8 core scalars / V

Key algebra: cost[k,t] = base[k,t] + (lse_k - ps_k) with
base = |ot-tt| + |oa-at| - logit[true class]; the (lse_k - ps_k) part is
independent of the assignment, so the 24-perm min runs on `base` alone and
sum_k lse_k + sum_k softplus(ps_k) = ln(prod_k se_k * prod_k (1+e^{ps_k}))
needs a single Ln per window.

Measured hardware model (perfetto): the input stream runs at ~250 GB/s
effective, ACT exp at ~1 col/cycle -- the two are nearly balanced, so the
kernel streams chunks DMA->exp->DVE trees with everything else hidden.
Hard-won trace lessons baked in: every elementwise operand is a FLAT
(P, W) AP (strided few-element runs cost ~1us regardless of size on any
engine); each logits chunk is its own contiguous DRAM tensor (sequential
HBM reads, ~25% faster than strided); per-chunk logits are packed k-major
so the se -> prod_k pipeline slices stay contiguous; DVE min/max run the
slow path (~3ns/col) so both abs-maxes share one wide op, as do the two
pair-mins; the final scalar is partition-summed on the idle PE so the
output DMA is 4 bytes on one queue (one completion event, not 16 -- events
drain at ~340ns each); one combined exp+ln act table (set 6) loads once up
front so no reload precedes the tail Ln.
"""
import numpy as np

L, B, K, C = 2048, 64, 4, 128
I = 512
NCORES = 8
BS = B // NCORES          # batch columns per core
NS = I * BS               # windows per core
P = 128                   # partitions
NT = NS // P              # 32 row-tiles per core
KC = K * C                # 512

# tiles per logits DMA chunk: tiny head (fast ramp), big middle, small drain
CHUNKS = [1, 2, 4, 8, 8, 6, 2, 1]
assert sum(CHUNKS) == NT

# small-tensor column offsets within the packed (P, SMW) fp16 tensor
OFF_OLT, OFF_OT, OFF_TT, OFF_OA, OFF_AT, OFF_PS, OFF_M, SMW = \
    0, 512, 640, 768, 896, 1024, 1152, 1184

# unordered window pairs; split q assigns A-pair PAIRS[SPLITS[q][0]] to
# outputs (k0,k1) and the complementary B-pair PAIRS[SPLITS[q][1]] to (k2,k3)
PAIRS = [(0, 1), (2, 3), (0, 2), (1, 3), (0, 3), (1, 2)]
SPLITS = [(0, 1), (1, 0), (2, 3), (3, 2), (4, 5), (5, 4)]

_PROGRAM = None


def _prep(in_time, in_amount, in_mcc, out_time, out_amount, out_logits,
          presence, lengths, indices, subset_lengths):
    """Host-side pure-index gather, mirroring reference _windows/_select."""
    f = np.float32
    idx = np.clip(np.asarray(indices), 0, L - 1)            # (I, B)
    br = np.arange(B)[None, :]
    win = (idx[:, :, None] + np.arange(K + 1)[None, None, :]) % L
    bw = br[:, :, None]
    tw = np.asarray(in_time)[win, bw].astype(f)             # (I,B,K+1)
    aw = np.asarray(in_amount)[win, bw].astype(f)
    cw = np.clip(np.asarray(in_mcc)[win, bw], 0, C - 1)     # (I,B,K+1)
    t_true = tw[..., 1:] - tw[..., :1]                      # (I,B,K)
    a_true = aw[..., 1:]
    true_c = cw[..., 1:]
    lg = np.asarray(out_logits)[idx, br].astype(f)          # (I,B,K,C)
    ol_true = np.take_along_axis(lg, true_c[:, :, None, :], axis=3)  # (I,B,K,T)
    ot = np.asarray(out_time)[idx, br].astype(f)            # (I,B,K)
    oa = np.asarray(out_amount)[idx, br].astype(f)
    ps = np.asarray(presence)[idx, br].astype(f)
    m = (np.arange(I)[:, None] < np.asarray(subset_lengths)[None, :]).astype(f)
    return dict(lg=lg, ol_true=ol_true, ot=ot, t_true=t_true, oa=oa,
                a_true=a_true, ps=ps, m=m)


def _pack_core(g, d):
    """Shard batch columns [d*BS, (d+1)*BS) and pack partition-major fp16:
    row n = i*BS + b_local lives at (tile j = n//P, partition p = n%P).
    Logits are split into per-chunk contiguous DRAM tensors, each packed
    k-major (P, (k, j_local, c)) so the per-chunk se/prod pipeline on the
    device slices contiguously."""
    sl = slice(d * BS, (d + 1) * BS)

    def pk(a):
        w = int(np.prod(a.shape[2:], dtype=np.int64)) if a.ndim > 2 else 1
        return a[:, sl].reshape(NT, P, w).transpose(1, 0, 2).reshape(P, NT * w)

    small = np.concatenate(
        [pk(g["ol_true"]), pk(g["ot"]), pk(g["t_true"]), pk(g["oa"]),
         pk(g["a_true"]), pk(g["ps"]), pk(g["m"])], axis=1).astype(np.float16)
    assert small.shape == (P, SMW)
    lg = g["lg"][:, sl].reshape(NT, P, K, C).astype(np.float16)  # (NT,P,K,C)
    out = {"small": small}
    off = 0
    for ci, t in enumerate(CHUNKS):
        ch = lg[off:off + t].transpose(1, 2, 0, 3)           # (P, K, t, C)
        out[f"lg{ci}"] = np.ascontiguousarray(ch.reshape(P, t * KC))
        off += t
    return out


def _build_program(debug=False):
    import concourse.bacc as bacc
    import concourse.tile as tile
    import concourse.mybir as mybir

    f32 = mybir.dt.float32
    f16 = mybir.dt.float16
    AF = mybir.ActivationFunctionType
    ALU = mybir.AluOpType
    AX = mybir.AxisListType.X

    nc = bacc.Bacc("TRN2", target_bir_lowering=False, debug=debug)
    lg_ds = [nc.dram_tensor(f"lg{ci}", [P, t * KC], f16, kind="ExternalInput")
             for ci, t in enumerate(CHUNKS)]
    sm_d = nc.dram_tensor("small", [P, SMW], f16, kind="ExternalInput")
    out_d = nc.dram_tensor("partial", [1, 1], f32, kind="ExternalOutput")

    NW = NT * K * K           # 512: flat width of (NT, K, K) tensors

    with tile.TileContext(nc) as tc:
        with tc.tile_pool(name="big", bufs=1) as big, \
             tc.tile_pool(name="res", bufs=1) as res, \
             tc.psum_pool(name="pacc", bufs=1) as pacc:

            def rtile(tag, shape, dt=f16):
                return res.tile(list(shape), dt, tag=tag, name=tag)

            # first logits chunk DMA on the ACT HWDGE queue: its descriptor
            # generation and completion run independently of the sync
            # queue's 8-deep descriptor backlog, so the exp stream starts
            # sooner; combined exp+ln table (set 6) loads right after
            lgs = [big.tile([P, t * KC], f16, tag=f"lg{ci}", name=f"lg{ci}")
                   for ci, t in enumerate(CHUNKS)]
            nc.scalar.dma_start(out=lgs[0][:], in_=lg_ds[0].ap())
            nc.scalar.add_instruction(mybir.InstLoadActFuncSet(
                name=nc.get_next_instruction_name(), ins=[], outs=[],
                act_func_set_id=6))
            sm = rtile("sm", (P, SMW))
            nc.sync.dma_start(out=sm[:], in_=sm_d.ap())
            for ci in range(1, len(CHUNKS)):
                nc.sync.dma_start(out=lgs[ci][:], in_=lg_ds[ci].ap())

            ones = rtile("ones", (P, 1), f32)
            nc.vector.memset(ones[:], 1.0)
            olt = sm[:, OFF_OLT:OFF_OT]                     # flat (P, 512)
            ot4 = sm[:, OFF_OT:OFF_TT].rearrange("p (j a) -> p j a", a=K)
            tt4 = sm[:, OFF_TT:OFF_OA].rearrange("p (j a) -> p j a", a=K)
            oa4 = sm[:, OFF_OA:OFF_AT].rearrange("p (j a) -> p j a", a=K)
            at4 = sm[:, OFF_AT:OFF_PS].rearrange("p (j a) -> p j a", a=K)
            ps4 = sm[:, OFF_PS:OFF_M].rearrange("p (j a) -> p j a", a=K)
            m1 = sm[:, OFF_M:SMW]

            TS = (P, NT, K, K)

            # --- base[n,k,t] = |ot-tt| + |oa-at| - olt: broadcast-heavy
            # subs on GpSimd into halves of one wide flat tile, both
            # abs-maxes as a single wide DVE op ---
            D = rtile("D", (P, 2 * NW))
            Dn = rtile("Dn", (P, 2 * NW))
            d_t4 = D[:, 0:NW].rearrange("p (j a b) -> p j a b", a=K, b=K)
            d_a4 = D[:, NW:].rearrange("p (j a b) -> p j a b", a=K, b=K)
            d_tn4 = Dn[:, 0:NW].rearrange("p (j a b) -> p j a b", a=K, b=K)
            d_an4 = Dn[:, NW:].rearrange("p (j a b) -> p j a b", a=K, b=K)
            nc.gpsimd.tensor_sub(d_t4, ot4.unsqueeze(3).broadcast_to(TS),
                                 tt4.unsqueeze(2).broadcast_to(TS))
            nc.gpsimd.tensor_sub(d_tn4, tt4.unsqueeze(2).broadcast_to(TS),
                                 ot4.unsqueeze(3).broadcast_to(TS))
            nc.gpsimd.tensor_sub(d_a4, oa4.unsqueeze(3).broadcast_to(TS),
                                 at4.unsqueeze(2).broadcast_to(TS))
            nc.gpsimd.tensor_sub(d_an4, at4.unsqueeze(2).broadcast_to(TS),
                                 oa4.unsqueeze(3).broadcast_to(TS))
            nc.vector.tensor_max(D[:], D[:], Dn[:])
            base = rtile("base", (P, NW))
            nc.vector.tensor_add(base[:], D[:, 0:NW], D[:, NW:])
            nc.vector.tensor_sub(base[:], base[:], olt)
            base4 = base[:].rearrange("p (j a b) -> p j a b", a=K, b=K)

            # pair sums A[t0,t1] = base[k0,t0]+base[k1,t1] (B for k2,k3)
            # and their transposes, packed as halves of two wide tiles so
            # the unordered-pair min is one wide contiguous DVE op
            b0 = base4[:, :, 0, :]
            b1 = base4[:, :, 1, :]
            b2 = base4[:, :, 2, :]
            b3 = base4[:, :, 3, :]
            AB = rtile("AB", (P, 2 * NW))
            ABt = rtile("ABt", (P, 2 * NW))
            A4 = AB[:, 0:NW].rearrange("p (j a b) -> p j a b", a=K, b=K)
            B4 = AB[:, NW:].rearrange("p (j a b) -> p j a b", a=K, b=K)
            At4 = ABt[:, 0:NW].rearrange("p (j a b) -> p j a b", a=K, b=K)
            Bt4 = ABt[:, NW:].rearrange("p (j a b) -> p j a b", a=K, b=K)
            nc.gpsimd.tensor_add(A4, b0.unsqueeze(3).broadcast_to(TS),
                                 b1.unsqueeze(2).broadcast_to(TS))
            nc.gpsimd.tensor_add(At4, b0.unsqueeze(2).broadcast_to(TS),
                                 b1.unsqueeze(3).broadcast_to(TS))
            nc.gpsimd.tensor_add(B4, b2.unsqueeze(3).broadcast_to(TS),
                                 b3.unsqueeze(2).broadcast_to(TS))
            nc.gpsimd.tensor_add(Bt4, b2.unsqueeze(2).broadcast_to(TS),
                                 b3.unsqueeze(3).broadcast_to(TS))

            # leftover pieces (tiny): e4 = exp(ps)+1, qe = prod_k e4,
            # pss = sum_k ps (pairwise; GpSimd can't free-dim reduce)
            e4 = rtile("e4", (P, NT, K))
            nc.scalar.activation(out=e4[:], in_=ps4, func=AF.Exp)
            nc.gpsimd.tensor_add(e4[:], e4[:],
                                 ones[:].unsqueeze(2).broadcast_to((P, NT, K)))
            q1 = rtile("q1", (P, NT, 2))
            nc.gpsimd.tensor_mul(q1[:], e4[:, :, 0:2], e4[:, :, 2:4])
            qe = rtile("qe", (P, NT), f32)
            nc.gpsimd.tensor_mul(qe[:], q1[:, :, 0], q1[:, :, 1])
            ps1 = rtile("ps1", (P, NT, 2), f32)
            nc.gpsimd.tensor_add(ps1[:], ps4[:, :, 0:2], ps4[:, :, 2:4])
            pss = rtile("pss", (P, NT), f32)
            nc.gpsimd.tensor_add(pss[:], ps1[:, :, 0], ps1[:, :, 1])

            mAB = rtile("mAB", (P, 2 * NW))
            mA4 = mAB[:, 0:NW].rearrange("p (j a b) -> p j a b", a=K, b=K)
            mB4 = mAB[:, NW:].rearrange("p (j a b) -> p j a b", a=K, b=K)
            pmin = rtile("pmin", (P, NT), f32)
            V6 = rtile("V6", (P, NT, 6))
            tot0 = rtile("tot0", (P, NT), f32)

            def emit_mins():
                # one wide contiguous min covers both pair tensors
                nc.vector.tensor_tensor(out=mAB[:], in0=AB[:], in1=ABt[:],
                                        op=ALU.min)
                for q, (ja, jb) in enumerate(SPLITS):
                    a0, a1 = PAIRS[ja]
                    c0, c1 = PAIRS[jb]
                    nc.gpsimd.tensor_add(V6[:, :, q], mA4[:, :, a0, a1],
                                         mB4[:, :, c0, c1])

            # --- per chunk: exp (ACT) -> halving-tree sums (DVE, packed
            # fp16 2x) -> qs -> qq (DVE, all contiguous thanks to k-major
            # chunk layout); the single Ln runs once at the end ---
            qq = rtile("qq", (P, NT), f32)
            off = 0
            for ci, t in enumerate(CHUNKS):
                lg = lgs[ci]
                nc.scalar.activation(out=lg[:], in_=lg[:], func=AF.Exp)
                g = t * K
                v = lg[:].rearrange("p (g c) -> p g c", c=C)
                h1 = big.tile([P, g, 64], f16, tag=f"h1_{ci}", name=f"h1_{ci}")
                nc.vector.tensor_add(h1[:], v[:, :, 0:64], v[:, :, 64:128])
                h2 = big.tile([P, g, 32], f16, tag=f"h2_{ci}", name=f"h2_{ci}")
                nc.vector.tensor_add(h2[:], h1[:, :, 0:32], h1[:, :, 32:64])
                h3 = big.tile([P, g, 16], f16, tag=f"h3_{ci}", name=f"h3_{ci}")
                nc.vector.tensor_add(h3[:], h2[:, :, 0:16], h2[:, :, 16:32])
                # g is (k, j_local) thanks to k-major packing, so se and
                # the pairwise products below slice contiguously
                se = big.tile([P, K, t], f16, tag=f"se_{ci}", name=f"se_{ci}")
                with nc.allow_low_precision(reason="sumexp fits fp16"):
                    nc.vector.tensor_reduce(out=se[:], in_=h3[:], axis=AX,
                                            op=ALU.add)
                s1 = big.tile([P, 2, t], f32, tag=f"s1_{ci}", name=f"s1_{ci}")
                nc.vector.tensor_mul(s1[:], se[:, 0:2, :], se[:, 2:4, :])
                nc.vector.tensor_mul(qq[:, off:off + t], s1[:, 0, :],
                                     s1[:, 1, :])
                off += t
                if ci == 2:
                    emit_mins()
                if ci == 3:
                    # min over the 6 split assignments, then fold -pss on
                    # GpSimd: tot0 = pmin - pss (well before the tail)
                    nc.vector.tensor_reduce(out=pmin[:], in_=V6[:], axis=AX,
                                            op=ALU.min)
                    nc.gpsimd.tensor_sub(tot0[:], pmin[:], pss[:])

            # tail: qq *= qe, one bulk Ln, (lnq + tot0) * m, rowsum,
            # partition-sum on PE, 4-byte single-event DMA out
            nc.vector.tensor_mul(qq[:], qq[:], qe[:])
            lnq = rtile("lnq", (P, NT), f32)
            nc.scalar.activation(out=lnq[:], in_=qq[:], func=AF.Ln)
            nc.vector.tensor_add(lnq[:], lnq[:], tot0[:])
            totm = rtile("totm", (P, NT), f32)
            nc.vector.tensor_mul(totm[:], lnq[:], m1)
            rowsum = rtile("rowsum", (P, 1), f32)
            nc.vector.tensor_reduce(out=rowsum[:], in_=totm[:], axis=AX,
                                    op=ALU.add)
            acc = pacc.tile([1, 1], f32, tag="acc", name="acc")
            nc.tensor.matmul(out=acc[:], lhsT=ones[:], rhs=rowsum[:],
                             start=True, stop=True)
            scl = rtile("scl", (1, 1), f32)
            nc.vector.tensor_copy(out=scl[:], in_=acc[:])
            nc.sync.dma_start(out=out_d.ap(), in_=scl[:], single_packet=True)

    nc.compile()
    return nc


def _get_program():
    global _PROGRAM
    if _PROGRAM is None:
        _PROGRAM = _build_program()
    return _PROGRAM


def kernel(**inputs):
    g = _prep(**inputs)
    in_maps = [_pack_core(g, d) for d in range(NCORES)]
    nc = _get_program()
    from concourse.bass_utils import run_bass_kernel_spmd
    res = run_bass_kernel_spmd(nc, in_maps, list(range(NCORES)))
    total = sum(float(r["partial"][0, 0]) for r in res.results)
    V = g["m"].sum(dtype=np.float64)
    return np.asarray(np.float32(total) / np.float32(V))


# revision 15
# speedup vs baseline: 1.2645x; 1.1823x over previous
"""DeTPP assignment loss on Trainium2, data-parallel over batch across 8 NeuronCores.

Pipeline per core (B_shard = 8 batch columns, N_s = 512*8 = 4096 windows):
  host   : pure-index gathers (rolling windows, per-batch row selection,
           true-class logit pick), shard + pack fp16 partition-major layouts
  device : sum(exp) over C=128 (the memory-bound bulk: 4.2 MiB of gathered
           fp16 logits per core), L1/CE cost assembly, exact 24-permutation
           assignment min via pair-sum decomposition, softplus leftover,
           mask-weighted reduction to one scalar (partition-sum on PE)
  host   : sum 8 core scalars / V

Key algebra: cost[k,t] = base[k,t] + (lse_k - ps_k) with
base = |ot-tt| + |oa-at| - logit[true class]; the (lse_k - ps_k) part is
independent of the assignment, so the 24-perm min runs on `base` alone and
sum_k lse_k + sum_k softplus(ps_k) = ln(prod_k se_k * prod_k (1+e^{ps_k}))
needs a single Ln per window.

Measured hardware model (perfetto): the input stream runs at ~250 GB/s
effective, ACT exp at ~1 col/cycle -- the two are nearly balanced, so the
kernel streams chunks DMA->exp->DVE trees with everything else hidden.
Hard-won trace lessons baked in: every elementwise operand is a FLAT
(P, W) AP (strided few-element runs cost ~1us regardless of size on any
engine); each logits chunk is its own contiguous DRAM tensor (sequential
HBM reads, ~25% faster than strided); per-chunk logits are packed k-major
so the se -> prod_k pipeline slices stay contiguous; DVE min/max run the
slow path (~3ns/col) so both abs-maxes share one wide op, as do the two
pair-mins; the final scalar is partition-summed on the idle PE so the
output DMA is 4 bytes on one queue (one completion event, not 16 -- events
drain at ~340ns each); one combined exp+ln act table (set 6) loads once up
front so no reload precedes the tail Ln.
"""
import numpy as np

L, B, K, C = 2048, 64, 4, 128
I = 512
NCORES = 8
BS = B // NCORES          # batch columns per core
NS = I * BS               # windows per core
P = 128                   # partitions
NT = NS // P              # 32 row-tiles per core
KC = K * C                # 512

# tiles per logits DMA chunk: tiny head (fast ramp), big middle, small drain
CHUNKS = [1, 2, 4, 8, 8, 6, 2, 1]
assert sum(CHUNKS) == NT

# small-tensor column offsets within the packed (P, SMW) fp16 tensor
OFF_OLT, OFF_OT, OFF_TT, OFF_OA, OFF_AT, OFF_PS, OFF_M, SMW = \
    0, 512, 640, 768, 896, 1024, 1152, 1184

# unordered window pairs; split q assigns A-pair PAIRS[SPLITS[q][0]] to
# outputs (k0,k1) and the complementary B-pair PAIRS[SPLITS[q][1]] to (k2,k3)
PAIRS = [(0, 1), (2, 3), (0, 2), (1, 3), (0, 3), (1, 2)]
SPLITS = [(0, 1), (1, 0), (2, 3), (3, 2), (4, 5), (5, 4)]

_PROGRAM = None


def _prep(in_time, in_amount, in_mcc, out_time, out_amount, out_logits,
          presence, lengths, indices, subset_lengths):
    """Host-side pure-index gather, mirroring reference _windows/_select."""
    f = np.float32
    idx = np.clip(np.asarray(indices), 0, L - 1)            # (I, B)
    br = np.arange(B)[None, :]
    win = (idx[:, :, None] + np.arange(K + 1)[None, None, :]) % L
    bw = br[:, :, None]
    tw = np.asarray(in_time)[win, bw].astype(f)             # (I,B,K+1)
    aw = np.asarray(in_amount)[win, bw].astype(f)
    cw = np.clip(np.asarray(in_mcc)[win, bw], 0, C - 1)     # (I,B,K+1)
    t_true = tw[..., 1:] - tw[..., :1]                      # (I,B,K)
    a_true = aw[..., 1:]
    true_c = cw[..., 1:]
    lg = np.asarray(out_logits)[idx, br].astype(f)          # (I,B,K,C)
    ol_true = np.take_along_axis(lg, true_c[:, :, None, :], axis=3)  # (I,B,K,T)
    ot = np.asarray(out_time)[idx, br].astype(f)            # (I,B,K)
    oa = np.asarray(out_amount)[idx, br].astype(f)
    ps = np.asarray(presence)[idx, br].astype(f)
    m = (np.arange(I)[:, None] < np.asarray(subset_lengths)[None, :]).astype(f)
    return dict(lg=lg, ol_true=ol_true, ot=ot, t_true=t_true, oa=oa,
                a_true=a_true, ps=ps, m=m)


def _pack_core(g, d):
    """Shard batch columns [d*BS, (d+1)*BS) and pack partition-major fp16:
    row n = i*BS + b_local lives at (tile j = n//P, partition p = n%P).
    Logits are split into per-chunk contiguous DRAM tensors, each packed
    k-major (P, (k, j_local, c)) so the per-chunk se/prod pipeline on the
    device slices contiguously."""
    sl = slice(d * BS, (d + 1) * BS)

    def pk(a):
        w = int(np.prod(a.shape[2:], dtype=np.int64)) if a.ndim > 2 else 1
        return a[:, sl].reshape(NT, P, w).transpose(1, 0, 2).reshape(P, NT * w)

    def pk_km(a):
        # k-major packing (P, (k, j)): keeps the device-side leftover
        # chain (e4 products, ps sums) fully contiguous
        return a[:, sl].reshape(NT, P, K).transpose(1, 2, 0).reshape(P, NT * K)

    small = np.concatenate(
        [pk(g["ol_true"]), pk(g["ot"]), pk(g["t_true"]), pk(g["oa"]),
         pk(g["a_true"]), pk_km(g["ps"]), pk(g["m"])], axis=1).astype(np.float16)
    assert small.shape == (P, SMW)
    lg = g["lg"][:, sl].reshape(NT, P, K, C).astype(np.float16)  # (NT,P,K,C)
    out = {"small": small}
    off = 0
    for ci, t in enumerate(CHUNKS):
        ch = lg[off:off + t].transpose(1, 2, 0, 3)           # (P, K, t, C)
        out[f"lg{ci}"] = np.ascontiguousarray(ch.reshape(P, t * KC))
        off += t
    return out


def _build_program(debug=False):
    import concourse.bacc as bacc
    import concourse.tile as tile
    import concourse.mybir as mybir

    f32 = mybir.dt.float32
    f16 = mybir.dt.float16
    AF = mybir.ActivationFunctionType
    ALU = mybir.AluOpType
    AX = mybir.AxisListType.X

    nc = bacc.Bacc("TRN2", target_bir_lowering=False, debug=debug)
    lg_ds = [nc.dram_tensor(f"lg{ci}", [P, t * KC], f16, kind="ExternalInput")
             for ci, t in enumerate(CHUNKS)]
    sm_d = nc.dram_tensor("small", [P, SMW], f16, kind="ExternalInput")
    out_d = nc.dram_tensor("partial", [1, 1], f32, kind="ExternalOutput")

    NW = NT * K * K           # 512: flat width of (NT, K, K) tensors

    with tile.TileContext(nc) as tc:
        with tc.tile_pool(name="big", bufs=1) as big, \
             tc.tile_pool(name="res", bufs=1) as res, \
             tc.psum_pool(name="pacc", bufs=1) as pacc:

            def rtile(tag, shape, dt=f16):
                return res.tile(list(shape), dt, tag=tag, name=tag)

            # first logits chunk DMA on the ACT HWDGE queue: its descriptor
            # generation and completion run independently of the sync
            # queue's 8-deep descriptor backlog, so the exp stream starts
            # sooner; combined exp+ln table (set 6) loads right after
            nc.scalar.add_instruction(mybir.InstLoadActFuncSet(
                name=nc.get_next_instruction_name(), ins=[], outs=[],
                act_func_set_id=6))
            lgs = [big.tile([P, t * KC], f16, tag=f"lg{ci}", name=f"lg{ci}")
                   for ci, t in enumerate(CHUNKS)]
            nc.scalar.dma_start(out=lgs[0][:], in_=lg_ds[0].ap())
            sm = rtile("sm", (P, SMW))
            nc.sync.dma_start(out=sm[:], in_=sm_d.ap())
            for ci in range(1, len(CHUNKS)):
                nc.sync.dma_start(out=lgs[ci][:], in_=lg_ds[ci].ap())

            ones = rtile("ones", (P, 1), f32)
            nc.vector.memset(ones[:], 1.0)
            olt = sm[:, OFF_OLT:OFF_OT]                     # flat (P, 512)
            ot4 = sm[:, OFF_OT:OFF_TT].rearrange("p (j a) -> p j a", a=K)
            tt4 = sm[:, OFF_TT:OFF_OA].rearrange("p (j a) -> p j a", a=K)
            oa4 = sm[:, OFF_OA:OFF_AT].rearrange("p (j a) -> p j a", a=K)
            at4 = sm[:, OFF_AT:OFF_PS].rearrange("p (j a) -> p j a", a=K)
            psk = sm[:, OFF_PS:OFF_M]                       # k-major (P, 128)
            m1 = sm[:, OFF_M:SMW]

            TS = (P, NT, K, K)

            # --- base[n,k,t] = |ot-tt| + |oa-at| - olt: broadcast-heavy
            # subs on GpSimd into halves of one wide flat tile, both
            # abs-maxes as a single wide DVE op ---
            D = rtile("D", (P, 2 * NW))
            Dn = rtile("Dn", (P, 2 * NW))
            d_t4 = D[:, 0:NW].rearrange("p (j a b) -> p j a b", a=K, b=K)
            d_a4 = D[:, NW:].rearrange("p (j a b) -> p j a b", a=K, b=K)
            d_tn4 = Dn[:, 0:NW].rearrange("p (j a b) -> p j a b", a=K, b=K)
            d_an4 = Dn[:, NW:].rearrange("p (j a b) -> p j a b", a=K, b=K)
            nc.gpsimd.tensor_sub(d_t4, ot4.unsqueeze(3).broadcast_to(TS),
                                 tt4.unsqueeze(2).broadcast_to(TS))
            nc.gpsimd.tensor_sub(d_tn4, tt4.unsqueeze(2).broadcast_to(TS),
                                 ot4.unsqueeze(3).broadcast_to(TS))
            nc.gpsimd.tensor_sub(d_a4, oa4.unsqueeze(3).broadcast_to(TS),
                                 at4.unsqueeze(2).broadcast_to(TS))
            nc.gpsimd.tensor_sub(d_an4, at4.unsqueeze(2).broadcast_to(TS),
                                 oa4.unsqueeze(3).broadcast_to(TS))
            nc.vector.tensor_max(D[:], D[:], Dn[:])
            base = rtile("base", (P, NW))
            nc.vector.tensor_add(base[:], D[:, 0:NW], D[:, NW:])
            nc.vector.tensor_sub(base[:], base[:], olt)
            base4 = base[:].rearrange("p (j a b) -> p j a b", a=K, b=K)

            # pair sums A[t0,t1] = base[k0,t0]+base[k1,t1] (B for k2,k3)
            # and their transposes, packed as halves of two wide tiles so
            # the unordered-pair min is one wide contiguous DVE op
            b0 = base4[:, :, 0, :]
            b1 = base4[:, :, 1, :]
            b2 = base4[:, :, 2, :]
            b3 = base4[:, :, 3, :]
            AB = rtile("AB", (P, 2 * NW))
            ABt = rtile("ABt", (P, 2 * NW))
            A4 = AB[:, 0:NW].rearrange("p (j a b) -> p j a b", a=K, b=K)
            B4 = AB[:, NW:].rearrange("p (j a b) -> p j a b", a=K, b=K)
            At4 = ABt[:, 0:NW].rearrange("p (j a b) -> p j a b", a=K, b=K)
            Bt4 = ABt[:, NW:].rearrange("p (j a b) -> p j a b", a=K, b=K)
            nc.gpsimd.tensor_add(A4, b0.unsqueeze(3).broadcast_to(TS),
                                 b1.unsqueeze(2).broadcast_to(TS))
            nc.gpsimd.tensor_add(At4, b0.unsqueeze(2).broadcast_to(TS),
                                 b1.unsqueeze(3).broadcast_to(TS))
            nc.gpsimd.tensor_add(B4, b2.unsqueeze(3).broadcast_to(TS),
                                 b3.unsqueeze(2).broadcast_to(TS))
            nc.gpsimd.tensor_add(Bt4, b2.unsqueeze(2).broadcast_to(TS),
                                 b3.unsqueeze(3).broadcast_to(TS))

            # leftover pieces (tiny, all contiguous thanks to k-major ps):
            # e4 = exp(ps)+1, qe = prod_k e4, pss = sum_k ps -- on DVE,
            # keeping GpSimd's busy window as short as possible (GpSimd
            # activity degrades concurrent DVE ops ~3x via SBUF contention)
            e4 = rtile("e4", (P, NT * K))
            nc.scalar.activation(out=e4[:], in_=psk, func=AF.Exp)
            nc.vector.tensor_scalar_add(e4[:], e4[:], 1.0)
            q1 = rtile("q1", (P, 2 * NT))
            nc.vector.tensor_mul(q1[:], e4[:, 0:2 * NT], e4[:, 2 * NT:])
            qe = rtile("qe", (P, NT), f32)
            nc.vector.tensor_mul(qe[:], q1[:, 0:NT], q1[:, NT:])
            ps1 = rtile("ps1", (P, 2 * NT), f32)
            nc.vector.tensor_add(ps1[:], psk[:, 0:2 * NT], psk[:, 2 * NT:])
            pss = rtile("pss", (P, NT), f32)
            nc.vector.tensor_add(pss[:], ps1[:, 0:NT], ps1[:, NT:])

            mAB = rtile("mAB", (P, 2 * NW))
            mA4 = mAB[:, 0:NW].rearrange("p (j a b) -> p j a b", a=K, b=K)
            mB4 = mAB[:, NW:].rearrange("p (j a b) -> p j a b", a=K, b=K)
            pmin = rtile("pmin", (P, NT), f32)
            V6 = rtile("V6", (P, NT, 6))
            tot0 = rtile("tot0", (P, NT), f32)

            def emit_mins():
                # one wide contiguous min covers both pair tensors
                nc.vector.tensor_tensor(out=mAB[:], in0=AB[:], in1=ABt[:],
                                        op=ALU.min)
                for q, (ja, jb) in enumerate(SPLITS):
                    a0, a1 = PAIRS[ja]
                    c0, c1 = PAIRS[jb]
                    nc.gpsimd.tensor_add(V6[:, :, q], mA4[:, :, a0, a1],
                                         mB4[:, :, c0, c1])

            # --- per chunk: exp (ACT) -> halving-tree sums (DVE, packed
            # fp16 2x) -> qs -> qq (DVE, all contiguous thanks to k-major
            # chunk layout); the single Ln runs once at the end ---
            qq = rtile("qq", (P, NT), f32)
            off = 0
            for ci, t in enumerate(CHUNKS):
                lg = lgs[ci]
                nc.scalar.activation(out=lg[:], in_=lg[:], func=AF.Exp)
                g = t * K
                v = lg[:].rearrange("p (g c) -> p g c", c=C)
                h1 = big.tile([P, g, 64], f16, tag=f"h1_{ci}", name=f"h1_{ci}")
                nc.vector.tensor_add(h1[:], v[:, :, 0:64], v[:, :, 64:128])
                h2 = big.tile([P, g, 32], f16, tag=f"h2_{ci}", name=f"h2_{ci}")
                nc.vector.tensor_add(h2[:], h1[:, :, 0:32], h1[:, :, 32:64])
                h3 = big.tile([P, g, 16], f16, tag=f"h3_{ci}", name=f"h3_{ci}")
                nc.vector.tensor_add(h3[:], h2[:, :, 0:16], h2[:, :, 16:32])
                # g is (k, j_local) thanks to k-major packing, so se and
                # the pairwise products below slice contiguously
                se = big.tile([P, K, t], f16, tag=f"se_{ci}", name=f"se_{ci}")
                with nc.allow_low_precision(reason="sumexp fits fp16"):
                    nc.vector.tensor_reduce(out=se[:], in_=h3[:], axis=AX,
                                            op=ALU.add)
                s1 = big.tile([P, 2, t], f32, tag=f"s1_{ci}", name=f"s1_{ci}")
                nc.vector.tensor_mul(s1[:], se[:, 0:2, :], se[:, 2:4, :])
                nc.vector.tensor_mul(qq[:, off:off + t], s1[:, 0, :],
                                     s1[:, 1, :])
                off += t
                if ci == 2:
                    emit_mins()
                if ci == 3:
                    # min over the 6 split assignments, then fold -pss:
                    # tot0 = pmin - pss (well before the tail)
                    nc.vector.tensor_reduce(out=pmin[:], in_=V6[:], axis=AX,
                                            op=ALU.min)
                    nc.vector.tensor_sub(tot0[:], pmin[:], pss[:])

            # tail: qq *= qe, one bulk Ln, (lnq + tot0) * m, rowsum,
            # partition-sum on PE, 4-byte single-event DMA out
            nc.vector.tensor_mul(qq[:], qq[:], qe[:])
            lnq = rtile("lnq", (P, NT), f32)
            nc.scalar.activation(out=lnq[:], in_=qq[:], func=AF.Ln)
            nc.vector.tensor_add(lnq[:], lnq[:], tot0[:])
            totm = rtile("totm", (P, NT), f32)
            nc.vector.tensor_mul(totm[:], lnq[:], m1)
            rowsum = rtile("rowsum", (P, 1), f32)
            nc.vector.tensor_reduce(out=rowsum[:], in_=totm[:], axis=AX,
                                    op=ALU.add)
            acc = pacc.tile([1, 1], f32, tag="acc", name="acc")
            nc.tensor.matmul(out=acc[:], lhsT=ones[:], rhs=rowsum[:],
                             start=True, stop=True)
            scl = rtile("scl", (1, 1), f32)
            nc.vector.tensor_copy(out=scl[:], in_=acc[:])
            nc.sync.dma_start(out=out_d.ap(), in_=scl[:], single_packet=True)

    nc.compile()
    return nc


def _get_program():
    global _PROGRAM
    if _PROGRAM is None:
        _PROGRAM = _build_program()
    return _PROGRAM


def kernel(**inputs):
    g = _prep(**inputs)
    in_maps = [_pack_core(g, d) for d in range(NCORES)]
    nc = _get_program()
    from concourse.bass_utils import run_bass_kernel_spmd
    res = run_bass_kernel_spmd(nc, in_maps, list(range(NCORES)))
    total = sum(float(r["partial"][0, 0]) for r in res.results)
    V = g["m"].sum(dtype=np.float64)
    return np.asarray(np.float32(total) / np.float32(V))
